# revision 20
# baseline (speedup 1.0000x reference)
"""Trainium2 Bass kernel for gated-attention segment pooling (Attn_Net_Gated).

Computation (matches the reference):
  f_n   = l2-normalize rows of feature [N, D]
  a     = sigmoid(f_n @ Wa.T + ba);  b = tanh(f_n @ Wb.T + bb)   [N, L]
  s     = ((a*b) @ Wc.T + bc) / TEMP                              [N, 1]
  w     = segment-softmax(s, batch)  (B=64 sorted contiguous bags)
  out   = segment-sum(w * f_n)                                    [B, D]
  returns (out, w, f_n)

Sharding: bags 8k..8k+7 -> core k (batch is sorted so each core gets a
contiguous row range). No cross-core communication needed; host gathers.

Per-core single fused pass over feature tiles (memory-bound):
  DMA f tile -> sumsq (ACT Square+accum) -> Newton rsqrt (DVE) ->
  f_norm f32 (DMA out) + f_norm fp16 -> PE transpose -> Wa/Wb matmuls ->
  tanh-based sigmoid/tanh (single ACT table set) -> g = a*b ->
  scores via matmul(lhsT=g, rhs=Wc_col) (row-partition layout) -> exp ->
  P = onehot*e as matmul weights -> pooling + denom accumulate in PSUM.
Softmax uses unnormalized exp(s) (|s| <= sum|Wc| ~ 13, no overflow risk);
denominators are divided out at the end.
"""

import math
import os
import sys
from contextlib import ExitStack

import numpy as np

for _p in ("/opt/trn_rl_repo",):
    if _p not in sys.path and os.path.isdir(_p):
        sys.path.insert(0, _p)

import concourse.bass as bass
import concourse.mybir as mybir
from concourse import bacc, tile
from concourse.bass_utils import run_bass_kernel_spmd

F32 = mybir.dt.float32
F16 = mybir.dt.float16
I32 = mybir.dt.int32
AF = mybir.ActivationFunctionType
ALU = mybir.AluOpType

N_CORES = 8
D = 1024
L = 128
B = 64
BAGS_PER_CORE = B // N_CORES
TEMP = 1.0
NCHUNK = D // 128  # 8 d-chunks per row tile
RSQRT_MAGIC = 0x5F3759DF


def build_program(T: int, G: int = 8):
    """Build the per-core Bass program for T row-tiles (128 rows each)."""
    nc = bacc.Bacc("TRN2", target_bir_lowering=False)
    R = T * 128

    # const blob layout (fp16): wa | wb | wc | ident | ones | per-tile onehot
    C_WA, C_WB = 0, D
    C_WC = 2 * D
    C_ID = C_WC + 1
    C_ONE = C_ID + 128
    C_OH = C_ONE + 1
    C16 = C_OH + BAGS_PER_CORE * T

    feat = nc.dram_tensor("feat", [R, D], F32, kind="ExternalInput")
    cst16 = nc.dram_tensor("cst16", [128, C16], F16, kind="ExternalInput")
    cst32 = nc.dram_tensor("cst32", [128, 3], F32, kind="ExternalInput")
    oht = nc.dram_tensor("oht", [BAGS_PER_CORE, R], F32, kind="ExternalInput")

    fnorm_out = nc.dram_tensor("fnorm_out", [R, D], F32, kind="ExternalOutput")
    score_out = nc.dram_tensor("score_out", [128, T], F32, kind="ExternalOutput")
    pool_out = nc.dram_tensor("pool_out", [BAGS_PER_CORE, D], F32, kind="ExternalOutput")

    n_groups = (T + G - 1) // G

    with tile.TileContext(nc) as tc:
        with ExitStack() as ctx:
            ec = ctx.enter_context
            cpool = ec(tc.tile_pool(name="const", bufs=1))
            pf = ec(tc.tile_pool(name="fin", bufs=10))
            psq = ec(tc.tile_pool(name="sq", bufs=2))
            pfn32 = ec(tc.tile_pool(name="fn32", bufs=4))
            pfn16 = ec(tc.tile_pool(name="fn16", bufs=G + 2))
            pftb = ec(tc.tile_pool(name="ftb", bufs=2))
            pnewt = ec(tc.tile_pool(name="newt", bufs=2))
            pgsb = ec(tc.tile_pool(name="gsb", bufs=2))
            psmall = ec(tc.tile_pool(name="psmall", bufs=4))
            pohtc = ec(tc.tile_pool(name="ohtc", bufs=2))
            ps_ft = ec(tc.tile_pool(name="ps_ft", bufs=2, space="PSUM"))
            ps_a = ec(tc.tile_pool(name="ps_a", bufs=1, space="PSUM"))
            ps_b = ec(tc.tile_pool(name="ps_b", bufs=1, space="PSUM"))
            ps_s = ec(tc.tile_pool(name="ps_s", bufs=1, space="PSUM"))
            ps_pool = ec(tc.tile_pool(name="ps_pool", bufs=1, space="PSUM"))
            ps_den = ec(tc.tile_pool(name="ps_den", bufs=1, space="PSUM"))
            # ---- constants (2 DMAs so const readers wait on one queue) ----
            c16_sb = cpool.tile([128, C16], F16)
            c32_sb = cpool.tile([128, 3], F32)
            nc.sync.dma_start(c16_sb[:], cst16[:])
            nc.sync.dma_start(c32_sb[:], cst32[:])
            wa_sb = c16_sb[:, C_WA:C_WA + D]
            wb_sb = c16_sb[:, C_WB:C_WB + D]
            wc_sb = c16_sb[:, C_WC:C_WC + 1]
            id_sb = c16_sb[:, C_ID:C_ID + 128]
            on_sb = c16_sb[:, C_ONE:C_ONE + 1]
            ba_sb = c32_sb[:, 0:1]
            bb_sb = c32_sb[:, 1:2]
            bc_sb = c32_sb[:, 2:3]
            e2_sb = cpool.tile([128, T], F32)
            sc_sb = cpool.tile([128, T], F32)

            pool_ps = ps_pool.tile([BAGS_PER_CORE, D], F32)
            den_ps = ps_den.tile([BAGS_PER_CORE, 1], F32)

            # ---- main fused loop over groups of up to G row-tiles ----
            for g in range(n_groups):
                t0 = g * G
                gi = min(G, T - t0)

                # load + per-row sum of squares
                f_list = []
                ss = pnewt.tile([128, G], F32, tag="ss")
                for j in range(gi):
                    t = t0 + j
                    f_t = pf.tile([128, D], F32)
                    nc.sync.dma_start(f_t[:], feat[t * 128:(t + 1) * 128, :])
                    sq = psq.tile([128, D], F16)
                    nc.scalar.activation(sq[:], f_t[:], AF.Square,
                                         accum_out=ss[:, j:j + 1])
                    f_list.append(f_t)

                # Newton rsqrt of ss -> rn (seed via int bit-trick; 3 iters)
                # clamp: zero-padded rows would overflow the iteration
                nc.vector.tensor_scalar_max(ss[:, :gi], ss[:, :gi], 1e-12)
                yi = pnewt.tile([128, G], I32, tag="yi")
                nc.vector.tensor_scalar(yi[:, :gi], ss[:, :gi].bitcast(I32),
                                        1, 0, op0=ALU.logical_shift_right,
                                        op1=ALU.bitwise_not)
                nc.vector.tensor_scalar_add(yi[:, :gi], yi[:, :gi], RSQRT_MAGIC + 1)
                y = yi[:].bitcast(F32)
                for _ in range(3):
                    y2 = pnewt.tile([128, G], F32, tag="y2")
                    nc.vector.tensor_tensor(y2[:, :gi], y[:, :gi], y[:, :gi], ALU.mult)
                    nc.vector.tensor_tensor(y2[:, :gi], y2[:, :gi], ss[:, :gi], ALU.mult)
                    nc.vector.tensor_scalar(y2[:, :gi], y2[:, :gi], -0.5, 1.5,
                                            op0=ALU.mult, op1=ALU.add)
                    yn = pnewt.tile([128, G], F32, tag="yn")
                    nc.vector.tensor_tensor(yn[:, :gi], y[:, :gi], y2[:, :gi], ALU.mult)
                    y = yn[:]

                # normalize (f32 out to HBM, f16 copy for matmuls)
                fn16_list = []
                for j in range(gi):
                    t = t0 + j
                    fn32 = pfn32.tile([128, D], F32)
                    nc.vector.tensor_scalar_mul(fn32[:], f_list[j][:], y[:, j:j + 1])
                    nc.sync.dma_start(fnorm_out[t * 128:(t + 1) * 128, :], fn32[:])
                    fn16 = pfn16.tile([128, D], F16)
                    nc.vector.tensor_scalar_mul(fn16[:], f_list[j][:], y[:, j:j + 1])
                    fn16_list.append(fn16)

                s_ps = ps_s.tile([128, G], F32, tag="s")

                # blocks of up to 4 tiles: transpose + Wa/Wb matmuls + gates
                nblk = (gi + 3) // 4
                for bi in range(nblk):
                    j0 = bi * 4
                    nb = min(4, gi - j0)
                    ftb = pftb.tile([128, 4 * D], F16)
                    for tt in range(nb):
                        for h in range(2):
                            ftp = ps_ft.tile([128, D // 2], F16)
                            for c in range(NCHUNK // 2):
                                ch = h * (NCHUNK // 2) + c
                                nc.tensor.transpose(
                                    ftp[:, c * 128:(c + 1) * 128],
                                    fn16_list[j0 + tt][:, ch * 128:(ch + 1) * 128],
                                    id_sb,
                                )
                            # psum -> sbuf, alternate DVE/ACT to balance engines
                            dst = ftb[:, tt * D + h * (D // 2):tt * D + (h + 1) * (D // 2)]
                            if h == 0:
                                nc.vector.tensor_copy(dst, ftp[:])
                            else:
                                nc.scalar.copy(dst, ftp[:])

                    a_ps = ps_a.tile([128, 512], F32)
                    b_ps = ps_b.tile([128, 512], F32)
                    nbr = nb * 128
                    # rhs AP: [p, (tt, c*128+r)] -> gather chunk c across tiles
                    ftb_v = ftb[:].rearrange("p (tt c r) -> p tt c r",
                                             tt=4, c=NCHUNK, r=128)
                    for c in range(NCHUNK):
                        nc.tensor.matmul(a_ps[:, :nbr], wa_sb[:, c * 128:(c + 1) * 128],
                                         ftb_v[:, :nb, c, :],
                                         start=(c == 0), stop=(c == NCHUNK - 1))
                    for c in range(NCHUNK):
                        nc.tensor.matmul(b_ps[:, :nbr], wb_sb[:, c * 128:(c + 1) * 128],
                                         ftb_v[:, :nb, c, :],
                                         start=(c == 0), stop=(c == NCHUNK - 1))

                    ta = pgsb.tile([128, 512], F16, tag="ta")
                    tb = pgsb.tile([128, 512], F16, tag="tb")
                    # sigmoid(x) = 0.5 + 0.5*tanh(x/2); exp/tanh share one table set
                    nc.scalar.activation(ta[:, :nbr], a_ps[:, :nbr], AF.Tanh,
                                         bias=ba_sb, scale=0.5)
                    nc.scalar.activation(tb[:, :nbr], b_ps[:, :nbr], AF.Tanh,
                                         bias=bb_sb, scale=1.0)
                    # g' = (ta + 1) * tb = 2*sigmoid_gate*tanh_gate; the 0.5
                    # is folded into wc on the host.
                    gt = pgsb.tile([128, 512], F16, tag="gt")
                    nc.vector.scalar_tensor_tensor(gt[:, :nbr], ta[:, :nbr], 1.0,
                                                   tb[:, :nbr], op0=ALU.add,
                                                   op1=ALU.mult)
                    for tt in range(nb):
                        nc.tensor.matmul(s_ps[:, j0 + tt:j0 + tt + 1],
                                         gt[:, tt * 128:(tt + 1) * 128], wc_sb,
                                         start=True, stop=True)

                # e = exp(s/TEMP + bc/TEMP)
                nc.scalar.activation(e2_sb[:, t0:t0 + gi], s_ps[:, :gi], AF.Exp,
                                     bias=bc_sb, scale=1.0 / TEMP)

                # pooling: P = onehot(bag) * e as stationary weights
                for j in range(gi):
                    t = t0 + j
                    ohc = C_OH + t * BAGS_PER_CORE
                    pt = psmall.tile([128, BAGS_PER_CORE], F16, tag="pt")
                    nc.vector.tensor_scalar(pt[:],
                                            c16_sb[:, ohc:ohc + BAGS_PER_CORE],
                                            e2_sb[:, t:t + 1], None, op0=ALU.mult)
                    st = (t == 0)
                    sp = (t == T - 1)
                    nc.tensor.matmul(pool_ps[:, 0:512], pt[:], fn16_list[j][:, 0:512],
                                     start=st, stop=sp, skip_group_check=True)
                    nc.tensor.matmul(pool_ps[:, 512:D], pt[:], fn16_list[j][:, 512:D],
                                     start=st, stop=sp, skip_group_check=True)
                    nc.tensor.matmul(den_ps[:], pt[:], on_sb,
                                     start=st, stop=sp, skip_group_check=True)

            # ---- finalize: denominators, pooled output, per-row scores ----
            dr = psmall.tile([BAGS_PER_CORE, 1], F32, tag="dr")
            nc.vector.reciprocal(dr[:], den_ps[:])
            pool_sb = cpool.tile([BAGS_PER_CORE, D], F32)
            nc.vector.tensor_scalar_mul(pool_sb[:], pool_ps[:], dr[:])
            nc.sync.dma_start(pool_out[:], pool_sb[:])

            for q in range((T + 3) // 4):
                q0 = q * 4
                nq = min(4, T - q0)
                ohc = pohtc.tile([BAGS_PER_CORE, 512], F32)
                nc.sync.dma_start(ohc[:, :nq * 128],
                                  oht[:, q0 * 128:(q0 + nq) * 128])
                dps = ps_s.tile([128, G], F32, tag="s")
                for tt in range(nq):
                    nc.tensor.matmul(dps[:, tt:tt + 1],
                                     ohc[:, tt * 128:(tt + 1) * 128], dr[:],
                                     start=True, stop=True)
                nc.vector.tensor_tensor(sc_sb[:, q0:q0 + nq], e2_sb[:, q0:q0 + nq],
                                        dps[:, :nq], ALU.mult)
            nc.sync.dma_start(score_out[:], sc_sb[:])

    return nc


def build_cst16(Wa, Wb, Wc, li, T):
    """Pack fp16 constants: wa | wb | wc | identity | ones | per-tile onehot."""
    def weight_sb(W):
        # [p, c*128+m] = W[m, c*128+p]  (lhsT chunks for contraction over D)
        wt = W.T.astype(np.float16)  # [D, L]
        return np.ascontiguousarray(
            wt.reshape(NCHUNK, 128, L).transpose(1, 0, 2).reshape(128, NCHUNK * L))

    # oh[p, t*8+b] = 1 if local bag of row (t*128+p) == b
    lit = li.reshape(T, 128).T  # [128, T]
    oh = (lit[:, :, None] ==
          np.arange(BAGS_PER_CORE, dtype=np.float32)[None, None, :])
    oh = oh.astype(np.float16).reshape(128, T * BAGS_PER_CORE)
    return np.ascontiguousarray(np.concatenate([
        weight_sb(Wa),
        weight_sb(Wb),
        (0.5 * Wc).reshape(L, 1).astype(np.float16),
        np.eye(128, dtype=np.float16),
        np.ones((128, 1), dtype=np.float16),
        oh,
    ], axis=1))


def prepare_host(inputs):
    """Shard inputs across cores; build all derived per-core arrays."""
    feature = np.ascontiguousarray(np.asarray(inputs["feature"], dtype=np.float32))
    batch = np.asarray(inputs["batch"]).astype(np.int64)
    Wa = np.asarray(inputs["Wa"], dtype=np.float32)
    ba = np.asarray(inputs["ba"], dtype=np.float32)
    Wb = np.asarray(inputs["Wb"], dtype=np.float32)
    bb = np.asarray(inputs["bb"], dtype=np.float32)
    Wc = np.asarray(inputs["Wc"], dtype=np.float32)
    bc = np.asarray(inputs["bc"], dtype=np.float32)

    n = feature.shape[0]
    counts = np.bincount(batch, minlength=B)
    cum = np.concatenate([[0], np.cumsum(counts)])
    starts = [int(cum[k * BAGS_PER_CORE]) for k in range(N_CORES)]
    ends = [int(cum[(k + 1) * BAGS_PER_CORE]) for k in range(N_CORES)]
    rmax = max(e - s for s, e in zip(starts, ends))
    rpad = ((rmax + 255) // 256) * 256
    T = rpad // 128

    cst32 = np.stack([ba / 2.0, bb, np.full(L, bc[0] / TEMP, np.float32)],
                     axis=1).astype(np.float32)
    cst32 = np.ascontiguousarray(cst32)  # [128, 3]

    in_maps = []
    for k in range(N_CORES):
        s, e = starts[k], ends[k]
        rk = e - s
        feat_k = np.zeros((rpad, D), dtype=np.float32)
        feat_k[:rk] = feature[s:e]
        li = np.full(rpad, -1.0, dtype=np.float32)
        li[:rk] = (batch[s:e] - k * BAGS_PER_CORE).astype(np.float32)
        oht = (li[None, :] ==
               np.arange(BAGS_PER_CORE, dtype=np.float32)[:, None]).astype(np.float32)
        m = {
            "feat": feat_k,
            "cst16": build_cst16(Wa, Wb, Wc, li, T),
            "cst32": cst32,
            "oht": np.ascontiguousarray(oht),
        }
        in_maps.append(m)

    meta = {"starts": starts, "ends": ends, "rpad": rpad, "T": T, "n": n}
    return in_maps, meta


def gather_outputs(results, meta, n):
    out = np.zeros((B, D), dtype=np.float32)
    score = np.zeros((n,), dtype=np.float32)
    feat_n = np.zeros((n, D), dtype=np.float32)
    for k in range(N_CORES):
        r = results[k]
        s, e = meta["starts"][k], meta["ends"][k]
        rk = e - s
        out[k * BAGS_PER_CORE:(k + 1) * BAGS_PER_CORE] = r["pool_out"]
        feat_n[s:e] = r["fnorm_out"][:rk]
        score[s:e] = np.ascontiguousarray(r["score_out"].T).reshape(-1)[:rk]
    return out, score[:, None], feat_n


def kernel(**inputs):
    in_maps, meta = prepare_host(inputs)
    nc = build_program(meta["T"])
    if not nc.is_finalized():
        nc.finalize()
    res = run_bass_kernel_spmd(nc, in_maps, core_ids=list(range(N_CORES)))
    return gather_outputs(res.results, meta, meta["n"])


if __name__ == "__main__":
    import reference
    inp = {k: np.asarray(v) for k, v in reference.setup_inputs().items()}
    outs = kernel(**inp)
    print([o.shape for o in outs])


# revision 27
# speedup vs baseline: 1.3372x; 1.3372x over previous
"""Trainium2 Bass kernel for gated-attention segment pooling (Attn_Net_Gated).

Computation (matches the reference):
  f_n   = l2-normalize rows of feature [N, D]
  a     = sigmoid(f_n @ Wa.T + ba);  b = tanh(f_n @ Wb.T + bb)   [N, L]
  s     = ((a*b) @ Wc.T + bc) / TEMP                              [N, 1]
  w     = segment-softmax(s, batch)  (B=64 sorted contiguous bags)
  out   = segment-sum(w * f_n)                                    [B, D]
  returns (out, w, f_n)

Sharding: bags 8k..8k+7 -> core k (batch is sorted so each core gets a
contiguous row range). No cross-core communication needed; host gathers.

Per-core single fused pass over feature tiles (memory-bound):
  DMA f tile -> sumsq (ACT Square+accum) -> Newton rsqrt (DVE) ->
  f_norm f32 (DMA out) + f_norm fp16 -> PE transpose -> Wa/Wb matmuls ->
  tanh-based sigmoid/tanh (single ACT table set) -> g = a*b ->
  scores via matmul(lhsT=g, rhs=Wc_col) (row-partition layout) -> exp ->
  P = onehot*e as matmul weights -> pooling + denom accumulate in PSUM.
Softmax uses unnormalized exp(s) (|s| <= sum|Wc| ~ 13, no overflow risk);
denominators are divided out at the end.
"""

import math
import os
import sys
from contextlib import ExitStack

import numpy as np

for _p in ("/opt/trn_rl_repo",):
    if _p not in sys.path and os.path.isdir(_p):
        sys.path.insert(0, _p)

import concourse.bass as bass
import concourse.mybir as mybir
from concourse import bacc, tile
from concourse.bass_utils import run_bass_kernel_spmd

F32 = mybir.dt.float32
F16 = mybir.dt.float16
I32 = mybir.dt.int32
AF = mybir.ActivationFunctionType
ALU = mybir.AluOpType

N_CORES = 8
D = 1024
L = 128
B = 64
BAGS_PER_CORE = B // N_CORES
TEMP = 1.0
NCHUNK = D // 128  # 8 d-chunks per row tile
RSQRT_MAGIC = 0x5F3759DF


def build_program(T: int, G: int = 8):
    """Build the per-core Bass program for T row-tiles (128 rows each)."""
    nc = bacc.Bacc("TRN2", target_bir_lowering=False)
    R = T * 128

    # const blob layout (fp16): wa | wb | wc | ident | ones | per-tile onehot
    C_WA, C_WB = 0, D
    C_WC = 2 * D
    C_ID = C_WC + 1
    C_ONE = C_ID + 128
    C_OH = C_ONE + 1
    C16 = C_OH + BAGS_PER_CORE * T

    feat = nc.dram_tensor("feat", [R, D], F32, kind="ExternalInput")
    cst16 = nc.dram_tensor("cst16", [128, C16], F16, kind="ExternalInput")
    cst32 = nc.dram_tensor("cst32", [128, 3], F32, kind="ExternalInput")
    oht = nc.dram_tensor("oht", [BAGS_PER_CORE, R], F16, kind="ExternalInput")

    fnorm_out = nc.dram_tensor("fnorm_out", [R, D], F32, kind="ExternalOutput")
    score_out = nc.dram_tensor("score_out", [128, T], F32, kind="ExternalOutput")
    pool_out = nc.dram_tensor("pool_out", [BAGS_PER_CORE, D], F32, kind="ExternalOutput")

    n_groups = (T + G - 1) // G

    with tile.TileContext(nc) as tc:
        with ExitStack() as ctx:
            ec = ctx.enter_context
            cpool = ec(tc.tile_pool(name="const", bufs=1))
            pf = ec(tc.tile_pool(name="fin", bufs=2 * G + 2))
            psq = ec(tc.tile_pool(name="sq", bufs=3))
            pfn32 = ec(tc.tile_pool(name="fn32", bufs=4))
            pfn16 = ec(tc.tile_pool(name="fn16", bufs=G + 2))
            pftb = ec(tc.tile_pool(name="ftb", bufs=2))
            pnewt = ec(tc.tile_pool(name="newt", bufs=3))
            pgsb = ec(tc.tile_pool(name="gsb", bufs=2))
            psmall = ec(tc.tile_pool(name="psmall", bufs=4))
            pohtc = ec(tc.tile_pool(name="ohtc", bufs=4))
            ps_ft = ec(tc.tile_pool(name="ps_ft", bufs=2, space="PSUM"))
            ps_a = ec(tc.tile_pool(name="ps_a", bufs=1, space="PSUM"))
            ps_b = ec(tc.tile_pool(name="ps_b", bufs=1, space="PSUM"))
            ps_s = ec(tc.tile_pool(name="ps_s", bufs=1, space="PSUM"))
            ps_pool = ec(tc.tile_pool(name="ps_pool", bufs=1, space="PSUM"))
            ps_den = ec(tc.tile_pool(name="ps_den", bufs=1, space="PSUM"))
            # ---- constants (2 DMAs so const readers wait on one queue) ----
            c16_sb = cpool.tile([128, C16], F16)
            c32_sb = cpool.tile([128, 3], F32)
            nc.sync.dma_start(c16_sb[:], cst16[:])
            nc.sync.dma_start(c32_sb[:], cst32[:])
            wa_sb = c16_sb[:, C_WA:C_WA + D]
            wb_sb = c16_sb[:, C_WB:C_WB + D]
            wc_sb = c16_sb[:, C_WC:C_WC + 1]
            id_sb = c16_sb[:, C_ID:C_ID + 128]
            on_sb = c16_sb[:, C_ONE:C_ONE + 1]
            ba_sb = c32_sb[:, 0:1]
            bb_sb = c32_sb[:, 1:2]
            bc_sb = c32_sb[:, 2:3]
            e2_sb = cpool.tile([128, T], F32)
            sc_sb = cpool.tile([128, T], F32)

            pool_ps = ps_pool.tile([BAGS_PER_CORE, D], F32)
            den_ps = ps_den.tile([BAGS_PER_CORE, 1], F32)

            # ---- main fused loop, software-pipelined one group ahead:
            # group g+1's loads+squares are emitted before group g's compute
            # so ACT/DMA stay busy while PE chews on the previous group.
            state = {}

            def emit_load_square(g):
                t0 = g * G
                gi = min(G, T - t0)
                f_list = []
                ss = pnewt.tile([128, G], F32, tag="ss", name=f"ss{g}")
                for j in range(gi):
                    t = t0 + j
                    f_t = pf.tile([128, D], F32, tag="f", name=f"f{t}")
                    nc.sync.dma_start(f_t[:], feat[t * 128:(t + 1) * 128, :])
                    sq = psq.tile([128, D], F16, tag="sq", name=f"sq{t}")
                    nc.scalar.activation(sq[:], f_t[:], AF.Square,
                                         accum_out=ss[:, j:j + 1])
                    f_list.append(f_t)
                state[g] = (f_list, ss)

            def emit_compute(g):
                t0 = g * G
                gi = min(G, T - t0)
                f_list, ss = state.pop(g)

                # Newton rsqrt of ss -> rn (seed via int bit-trick; 3 iters)
                # clamp: zero-padded rows would overflow the iteration
                nc.vector.tensor_scalar_max(ss[:, :gi], ss[:, :gi], 1e-12)
                yi = pnewt.tile([128, G], I32, tag="yi", name=f"yi{g}")
                nc.vector.tensor_scalar(yi[:, :gi], ss[:, :gi].bitcast(I32),
                                        1, 0, op0=ALU.logical_shift_right,
                                        op1=ALU.bitwise_not)
                nc.vector.tensor_scalar_add(yi[:, :gi], yi[:, :gi], RSQRT_MAGIC + 1)
                y = yi[:].bitcast(F32)
                for it in range(3):
                    y2 = pnewt.tile([128, G], F32, tag="y2", name=f"y2_{g}_{it}")
                    nc.vector.tensor_tensor(y2[:, :gi], y[:, :gi], y[:, :gi], ALU.mult)
                    nc.vector.tensor_tensor(y2[:, :gi], y2[:, :gi], ss[:, :gi], ALU.mult)
                    nc.vector.tensor_scalar(y2[:, :gi], y2[:, :gi], -0.5, 1.5,
                                            op0=ALU.mult, op1=ALU.add)
                    yn = pnewt.tile([128, G], F32, tag="yn", name=f"yn{g}_{it}")
                    nc.vector.tensor_tensor(yn[:, :gi], y[:, :gi], y2[:, :gi], ALU.mult)
                    y = yn[:]

                # normalize (f32 out to HBM, f16 copy for matmuls)
                fn16_list = []
                for j in range(gi):
                    t = t0 + j
                    fn32 = pfn32.tile([128, D], F32)
                    nc.vector.tensor_scalar_mul(fn32[:], f_list[j][:], y[:, j:j + 1])
                    nc.sync.dma_start(fnorm_out[t * 128:(t + 1) * 128, :], fn32[:])
                    fn16 = pfn16.tile([128, D], F16)
                    nc.vector.tensor_scalar_mul(fn16[:], f_list[j][:], y[:, j:j + 1])
                    fn16_list.append(fn16)

                s_ps = ps_s.tile([128, G], F32, tag="s")

                # blocks of up to 4 tiles: transpose + Wa/Wb matmuls + gates
                nblk = (gi + 3) // 4
                for bi in range(nblk):
                    j0 = bi * 4
                    nb = min(4, gi - j0)
                    ftb = pftb.tile([128, 4 * D], F16)
                    for tt in range(nb):
                        for h in range(2):
                            ftp = ps_ft.tile([128, D // 2], F16)
                            for c in range(NCHUNK // 2):
                                ch = h * (NCHUNK // 2) + c
                                nc.tensor.transpose(
                                    ftp[:, c * 128:(c + 1) * 128],
                                    fn16_list[j0 + tt][:, ch * 128:(ch + 1) * 128],
                                    id_sb,
                                )
                            # psum -> sbuf, alternate DVE/ACT to balance engines
                            dst = ftb[:, tt * D + h * (D // 2):tt * D + (h + 1) * (D // 2)]
                            if h == 0:
                                nc.vector.tensor_copy(dst, ftp[:])
                            else:
                                nc.scalar.copy(dst, ftp[:])

                    a_ps = ps_a.tile([128, 512], F32)
                    b_ps = ps_b.tile([128, 512], F32)
                    nbr = nb * 128
                    # rhs AP: [p, (tt, c*128+r)] -> gather chunk c across tiles
                    ftb_v = ftb[:].rearrange("p (tt c r) -> p tt c r",
                                             tt=4, c=NCHUNK, r=128)
                    for c in range(NCHUNK):
                        nc.tensor.matmul(a_ps[:, :nbr], wa_sb[:, c * 128:(c + 1) * 128],
                                         ftb_v[:, :nb, c, :],
                                         start=(c == 0), stop=(c == NCHUNK - 1))
                    for c in range(NCHUNK):
                        nc.tensor.matmul(b_ps[:, :nbr], wb_sb[:, c * 128:(c + 1) * 128],
                                         ftb_v[:, :nb, c, :],
                                         start=(c == 0), stop=(c == NCHUNK - 1))

                    ta = pgsb.tile([128, 512], F16, tag="ta")
                    tb = pgsb.tile([128, 512], F16, tag="tb")
                    # sigmoid(x) = 0.5 + 0.5*tanh(x/2); exp/tanh share one table set
                    nc.scalar.activation(ta[:, :nbr], a_ps[:, :nbr], AF.Tanh,
                                         bias=ba_sb, scale=0.5)
                    nc.scalar.activation(tb[:, :nbr], b_ps[:, :nbr], AF.Tanh,
                                         bias=bb_sb, scale=1.0)
                    # g' = (ta + 1) * tb = 2*sigmoid_gate*tanh_gate; the 0.5
                    # is folded into wc on the host.
                    gt = pgsb.tile([128, 512], F16, tag="gt")
                    nc.vector.scalar_tensor_tensor(gt[:, :nbr], ta[:, :nbr], 1.0,
                                                   tb[:, :nbr], op0=ALU.add,
                                                   op1=ALU.mult)
                    for tt in range(nb):
                        nc.tensor.matmul(s_ps[:, j0 + tt:j0 + tt + 1],
                                         gt[:, tt * 128:(tt + 1) * 128], wc_sb,
                                         start=True, stop=True)

                # e = exp(s/TEMP + bc/TEMP)
                nc.scalar.activation(e2_sb[:, t0:t0 + gi], s_ps[:, :gi], AF.Exp,
                                     bias=bc_sb, scale=1.0 / TEMP)

                # pooling: P = onehot(bag) * e as stationary weights
                for j in range(gi):
                    t = t0 + j
                    ohc = C_OH + t * BAGS_PER_CORE
                    pt = psmall.tile([128, BAGS_PER_CORE], F16, tag="pt")
                    nc.vector.tensor_scalar(pt[:],
                                            c16_sb[:, ohc:ohc + BAGS_PER_CORE],
                                            e2_sb[:, t:t + 1], None, op0=ALU.mult)
                    st = (t == 0)
                    sp = (t == T - 1)
                    nc.tensor.matmul(pool_ps[:, 0:512], pt[:], fn16_list[j][:, 0:512],
                                     start=st, stop=sp, skip_group_check=True)
                    nc.tensor.matmul(pool_ps[:, 512:D], pt[:], fn16_list[j][:, 512:D],
                                     start=st, stop=sp, skip_group_check=True)
                    nc.tensor.matmul(den_ps[:], pt[:], on_sb,
                                     start=st, stop=sp, skip_group_check=True)

            emit_load_square(0)
            for g in range(n_groups):
                if g + 1 < n_groups:
                    emit_load_square(g + 1)
                emit_compute(g)

            # ---- finalize: denominators, pooled output, per-row scores ----
            dr = psmall.tile([BAGS_PER_CORE, 1], F32, tag="dr")
            nc.vector.reciprocal(dr[:], den_ps[:])
            pool_sb = cpool.tile([BAGS_PER_CORE, D], F32)
            nc.vector.tensor_scalar_mul(pool_sb[:], pool_ps[:], dr[:])
            nc.sync.dma_start(pool_out[:], pool_sb[:])

            # fp16 hi/lo split of 1/denom keeps the gather matmuls in fp16
            # (fp32 matmuls lower to slow HIGH/LOW passes) at full precision
            drh = psmall.tile([BAGS_PER_CORE, 1], F16, tag="drh")
            nc.vector.tensor_copy(drh[:], dr[:])
            drl = psmall.tile([BAGS_PER_CORE, 1], F16, tag="drl")
            nc.vector.tensor_tensor(drl[:], dr[:], drh[:], ALU.subtract)

            for q in range((T + 3) // 4):
                q0 = q * 4
                nq = min(4, T - q0)
                ohc = pohtc.tile([BAGS_PER_CORE, 512], F16)
                nc.sync.dma_start(ohc[:, :nq * 128],
                                  oht[:, q0 * 128:(q0 + nq) * 128])
                dps = ps_s.tile([128, G], F32, tag="s")
                for tt in range(nq):
                    nc.tensor.matmul(dps[:, tt:tt + 1],
                                     ohc[:, tt * 128:(tt + 1) * 128], drh[:],
                                     start=True, stop=False)
                    nc.tensor.matmul(dps[:, tt:tt + 1],
                                     ohc[:, tt * 128:(tt + 1) * 128], drl[:],
                                     start=False, stop=True)
                nc.vector.tensor_tensor(sc_sb[:, q0:q0 + nq], e2_sb[:, q0:q0 + nq],
                                        dps[:, :nq], ALU.mult)
            nc.sync.dma_start(score_out[:], sc_sb[:])

    return nc


def build_cst16(Wa, Wb, Wc, li, T):
    """Pack fp16 constants: wa | wb | wc | identity | ones | per-tile onehot."""
    def weight_sb(W):
        # [p, c*128+m] = W[m, c*128+p]  (lhsT chunks for contraction over D)
        wt = W.T.astype(np.float16)  # [D, L]
        return np.ascontiguousarray(
            wt.reshape(NCHUNK, 128, L).transpose(1, 0, 2).reshape(128, NCHUNK * L))

    # oh[p, t*8+b] = 1 if local bag of row (t*128+p) == b
    lit = li.reshape(T, 128).T  # [128, T]
    oh = (lit[:, :, None] ==
          np.arange(BAGS_PER_CORE, dtype=np.float32)[None, None, :])
    oh = oh.astype(np.float16).reshape(128, T * BAGS_PER_CORE)
    return np.ascontiguousarray(np.concatenate([
        weight_sb(Wa),
        weight_sb(Wb),
        (0.5 * Wc).reshape(L, 1).astype(np.float16),
        np.eye(128, dtype=np.float16),
        np.ones((128, 1), dtype=np.float16),
        oh,
    ], axis=1))


def prepare_host(inputs):
    """Shard inputs across cores; build all derived per-core arrays."""
    feature = np.ascontiguousarray(np.asarray(inputs["feature"], dtype=np.float32))
    batch = np.asarray(inputs["batch"]).astype(np.int64)
    Wa = np.asarray(inputs["Wa"], dtype=np.float32)
    ba = np.asarray(inputs["ba"], dtype=np.float32)
    Wb = np.asarray(inputs["Wb"], dtype=np.float32)
    bb = np.asarray(inputs["bb"], dtype=np.float32)
    Wc = np.asarray(inputs["Wc"], dtype=np.float32)
    bc = np.asarray(inputs["bc"], dtype=np.float32)

    n = feature.shape[0]
    counts = np.bincount(batch, minlength=B)
    cum = np.concatenate([[0], np.cumsum(counts)])
    starts = [int(cum[k * BAGS_PER_CORE]) for k in range(N_CORES)]
    ends = [int(cum[(k + 1) * BAGS_PER_CORE]) for k in range(N_CORES)]
    rmax = max(e - s for s, e in zip(starts, ends))
    rpad = ((rmax + 255) // 256) * 256
    T = rpad // 128

    cst32 = np.stack([ba / 2.0, bb, np.full(L, bc[0] / TEMP, np.float32)],
                     axis=1).astype(np.float32)
    cst32 = np.ascontiguousarray(cst32)  # [128, 3]

    in_maps = []
    for k in range(N_CORES):
        s, e = starts[k], ends[k]
        rk = e - s
        feat_k = np.zeros((rpad, D), dtype=np.float32)
        feat_k[:rk] = feature[s:e]
        li = np.full(rpad, -1.0, dtype=np.float32)
        li[:rk] = (batch[s:e] - k * BAGS_PER_CORE).astype(np.float32)
        oht = (li[None, :] ==
               np.arange(BAGS_PER_CORE, dtype=np.float32)[:, None]).astype(np.float16)
        m = {
            "feat": feat_k,
            "cst16": build_cst16(Wa, Wb, Wc, li, T),
            "cst32": cst32,
            "oht": np.ascontiguousarray(oht),
        }
        in_maps.append(m)

    meta = {"starts": starts, "ends": ends, "rpad": rpad, "T": T, "n": n}
    return in_maps, meta


def gather_outputs(results, meta, n):
    out = np.zeros((B, D), dtype=np.float32)
    score = np.zeros((n,), dtype=np.float32)
    feat_n = np.zeros((n, D), dtype=np.float32)
    for k in range(N_CORES):
        r = results[k]
        s, e = meta["starts"][k], meta["ends"][k]
        rk = e - s
        out[k * BAGS_PER_CORE:(k + 1) * BAGS_PER_CORE] = r["pool_out"]
        feat_n[s:e] = r["fnorm_out"][:rk]
        score[s:e] = np.ascontiguousarray(r["score_out"].T).reshape(-1)[:rk]
    return out, score[:, None], feat_n


def kernel(**inputs):
    in_maps, meta = prepare_host(inputs)
    nc = build_program(meta["T"])
    if not nc.is_finalized():
        nc.finalize()
    res = run_bass_kernel_spmd(nc, in_maps, core_ids=list(range(N_CORES)))
    return gather_outputs(res.results, meta, meta["n"])


if __name__ == "__main__":
    import reference
    inp = {k: np.asarray(v) for k, v in reference.setup_inputs().items()}
    outs = kernel(**inp)
    print([o.shape for o in outs])
